# revision 34
# baseline (speedup 1.0000x reference)
"""Multi-head attention (RoPE + causal-mask softmax) on 8 TRN2 NeuronCores.

Sharding: batch x head-group (2 batches x 4 groups of 4 heads). Each core
computes q/k/v projections for its 4 heads over the full sequence and
attention for all 2048 queries. Per 512-query chunk, an AllGather over the
4 cores of the batch exchanges normalized attention outputs; each core then
reads its rank's 128-query column window (runtime-offset DMA) and runs the
full 16-head o_proj locally, so no partial-sum reduction is needed.

Head-sharding keeps the program SPMD-uniform while letting the causal
structure skip score blocks above the block diagonal (every core sees the
same query/key trapezoid). kernel() inspects the mask at runtime: if it is
(effectively) causal it builds the trapezoid program, otherwise a full-mask
fallback program.
"""

from contextlib import ExitStack

import numpy as np

import concourse.bass as bass
import concourse.tile as tile
from concourse import bacc, mybir
from concourse.alu_op_type import AluOpType
from concourse.bass_utils import run_bass_kernel_spmd

AF = mybir.ActivationFunctionType
F32 = mybir.dt.float32
F16 = mybir.dt.float16
BF16 = mybir.dt.bfloat16

B, S, HID, NH, HD = 2, 2048, 1024, 16, 64
SCALE = 1.0 / np.sqrt(HD)
N_CORES = 8
HPC = 4            # heads per core
CPB = 4            # cores per batch
HC = HID // 128    # hidden chunks (8)
QC = S // 512      # query chunks of 512 (4)
KC = S // 128      # key chunks of 128 (16)
GROUPS = [[0, 1, 2, 3], [4, 5, 6, 7]]


def build_program(causal: bool):
    nc = bacc.Bacc("TRN2", target_bir_lowering=False, debug=False,
                   num_devices=N_CORES)

    hsT = nc.dram_tensor("hsT", [HID, S], BF16, kind="ExternalInput").ap()
    cosk = nc.dram_tensor("cosk", [128, S], BF16, kind="ExternalInput").ap()
    sink = nc.dram_tensor("sink", [128, S], BF16, kind="ExternalInput").ap()
    # causal: exp(mask) diag blocks, [keys 512 per qc stacked, q 512 x2 dup]
    # general: exp(mask) full, [keys S, q S]
    em_cols = 1024 if causal else S
    emask = nc.dram_tensor("emask", [S, em_cols], F16, kind="ExternalInput").ap()
    wq = nc.dram_tensor("wq", [HID, HPC * HD], BF16, kind="ExternalInput").ap()
    wk = nc.dram_tensor("wk", [HID, HPC * HD], BF16, kind="ExternalInput").ap()
    wv = nc.dram_tensor("wv", [HID, HPC * HD], BF16, kind="ExternalInput").ap()
    wo = nc.dram_tensor("wo", [HID, HID], F16, kind="ExternalInput").ap()
    out = nc.dram_tensor("out", [512, HID], F32, kind="ExternalOutput").ap()

    with tile.TileContext(nc) as tc, ExitStack() as top:
        res = top.enter_context(tc.tile_pool(name="res", bufs=1))
        dram = top.enter_context(tc.tile_pool(name="dram", bufs=1, space="DRAM"))

        # AllGather exchange buffers, one per query chunk: each core
        # contributes its normalized [2 pairs x 128, 512 q] block; after the
        # gather, rows [g*256 + p*128] hold peer g's pair-p heads and every
        # core reads its own 128-query column window (rank-dynamic offset).
        ag_in = [dram.tile([256, 512], F16, tag=f"ai{qc}", name=f"ai{qc}")
                 for qc in range(QC)]
        ag_out = [dram.tile([CPB * 256, 512], F16, tag=f"ao{qc}",
                            name=f"ao{qc}") for qc in range(QC)]

        # ---- resident tiles, batched multi-dim DMA loads --------------------
        # hsT as one [128, HC*S] tile; per-seq-chunk loads spread across the
        # three DMA-capable queues so the K projection can start early.
        hsT_t = res.tile([128, HC * S], BF16, tag="hsT")
        hs3 = hsT_t[:].rearrange("p (c s) -> p c s", c=HC)
        wk_t = res.tile([128, HC * 256], BF16, tag="wk")
        nc.gpsimd.dma_start(wk_t[:].rearrange("p (c n) -> p c n", c=HC),
                            wk[:].rearrange("(c p) n -> p c n", c=HC))
        # first seq chunk split across two queues so K proj starts early
        nc.sync.dma_start(
            hs3[:, 0:4, 0:512],
            hsT[0:512, 0:512].rearrange("(c p) q -> p c q", c=4))
        nc.scalar.dma_start(
            hs3[:, 4:8, 0:512],
            hsT[512:1024, 0:512].rearrange("(c p) q -> p c q", c=4))
        for sc, eng in ((1, nc.sync), (2, nc.scalar), (3, nc.sync)):
            eng.dma_start(
                hs3[:, :, sc * 512:(sc + 1) * 512],
                hsT[:, sc * 512:(sc + 1) * 512].rearrange(
                    "(c p) q -> p c q", c=HC))

        def hsv(hc):
            return hsT_t[:, hc * S:(hc + 1) * S]

        wq_t = res.tile([128, HC * 256], BF16, tag="wq")
        nc.gpsimd.dma_start(wq_t[:].rearrange("p (c n) -> p c n", c=HC),
                            wq[:].rearrange("(c p) n -> p c n", c=HC))
        wv_t = res.tile([128, HC * 256], BF16, tag="wv")
        nc.gpsimd.dma_start(wv_t[:].rearrange("p (c n) -> p c n", c=HC),
                            wv[:].rearrange("(c p) n -> p c n", c=HC))
        cos_sb = res.tile([128, S], BF16, tag="cos")
        nc.scalar.dma_start(cos_sb[:], cosk[:])
        sin_sb = res.tile([128, S], BF16, tag="sin")
        nc.scalar.dma_start(sin_sb[:], sink[:])
        # K/Q pair-packed [head dims: pair head A 0:64, head B 64:128]
        kt = [res.tile([128, S], BF16, tag=f"kt{p}", name=f"kt{p}")
              for p in range(2)]
        qt = [res.tile([128, S], BF16, tag=f"qt{p}", name=f"qt{p}")
              for p in range(2)]
        # V augmented: per key-chunk, 4 heads x (64 cols + ones col)
        v_sb = [res.tile([128, HPC * 65], F16, tag=f"v{kc}", name=f"v{kc}")
                for kc in range(KC)]
        ones4 = res.tile([128, HPC], F16, tag="ones4")
        nc.gpsimd.memset(ones4[:], 1.0)
        # normalized attention output, pair-packed
        acc2 = [res.tile([128, S], F16, tag=f"acc2_{p}", name=f"acc2_{p}")
                for p in range(2)]

        # ---- K/Q projection + RoPE -----------------------------------------
        # Full-width rope per pair: project all 4 seq chunks into kraw, swap
        # the 32-row rotate-half blocks with 4 wide SBUF->SBUF DMAs, then 3
        # full-width vector ops. sin_sb carries rotate-half's sign.
        with tc.tile_pool(name="rope", bufs=2) as rope, \
             tc.tile_pool(name="psk", bufs=2, space="PSUM") as psk:
            for w_t, dst_l in ((wk_t, kt), (wq_t, qt)):
                for p in range(2):
                    with nc.allow_low_precision(reason="bf16 rope"):
                        kraw = rope.tile([128, S], BF16, tag="kraw")
                        for sc in range(4):
                            ps = psk.tile([128, 512], F32, tag="psk")
                            for hc in range(HC):
                                nc.tensor.matmul(
                                    ps[:],
                                    w_t[:, hc * 256 + p * 128:
                                        hc * 256 + (p + 1) * 128],
                                    hsv(hc)[:, sc * 512:(sc + 1) * 512],
                                    start=(hc == 0), stop=(hc == HC - 1))
                            nc.vector.tensor_copy(
                                kraw[:, sc * 512:(sc + 1) * 512], ps[:])
                        ksw = rope.tile([128, S], BF16, tag="ksw")
                        for hb in (0, 64):
                            nc.gpsimd.dma_start(ksw[hb:hb + 32, :],
                                                kraw[hb + 32:hb + 64, :])
                            nc.gpsimd.dma_start(ksw[hb + 32:hb + 64, :],
                                                kraw[hb:hb + 32, :])
                        t1 = rope.tile([128, S], BF16, tag="t1")
                        nc.vector.tensor_tensor(t1[:], kraw[:], cos_sb[:],
                                                AluOpType.mult)
                        t2 = rope.tile([128, S], BF16, tag="t2")
                        nc.vector.tensor_tensor(t2[:], ksw[:], sin_sb[:],
                                                AluOpType.mult)
                        nc.vector.tensor_tensor(dst_l[p][:], t1[:], t2[:],
                                                AluOpType.add)

        # ---- mid loads: wo + exp(mask), transfer during the rope window -----
        wo_t = res.tile([128, HC * HID], F16, tag="wo")
        nc.scalar.dma_start(wo_t[:].rearrange("p (c n) -> p c n", c=HC),
                            wo[:].rearrange("(c p) n -> p c n", c=HC))
        em_sb = []
        nkc_em = 4 if causal else KC
        em_w = 1024 if causal else 512
        for qc in range(QC):
            t = res.tile([128, nkc_em * em_w], F16, tag=f"em{qc}",
                         name=f"em{qc}")
            esrc = (emask[qc * 512:(qc + 1) * 512, :] if causal
                    else emask[:, qc * 512:(qc + 1) * 512])
            nc.scalar.dma_start(
                t[:].rearrange("p (c q) -> p c q", c=nkc_em),
                esrc.rearrange("(c p) q -> p c q", c=nkc_em))
            em_sb.append(t[:].rearrange("p (c q) -> p c q", c=nkc_em))

        # ---- V projection ---------------------------------------------------
        with tc.tile_pool(name="psv", bufs=2, space="PSUM") as psv:
            for kc in range(KC):
                ps = psv.tile([128, HPC * HD], F32, tag="psv")
                for hc in range(HC):
                    nc.tensor.matmul(
                        ps[:], hsv(hc)[:, kc * 128:(kc + 1) * 128],
                        wv_t[:, hc * 256:(hc + 1) * 256],
                        start=(hc == 0), stop=(hc == HC - 1))
                v3 = v_sb[kc][:].rearrange("p (h c) -> p h c", h=HPC)
                ps3 = ps[:].rearrange("p (h c) -> p h c", h=HPC)
                with nc.allow_low_precision(reason="fp16 v"):
                    nc.vector.tensor_copy(v3[:, :, 0:64], ps3[:])
                nc.gpsimd.tensor_copy(v3[:, :, 64], ones4[:])

        # ---- attention + exchange + o_proj, per query chunk -----------------
        with tc.tile_pool(name="expp", bufs=4) as expp, \
             tc.tile_pool(name="nrm", bufs=2) as nrm, \
             tc.tile_pool(name="gath", bufs=2) as gathp, \
             tc.tile_pool(name="outp", bufs=2) as outp, \
             tc.tile_pool(name="pss", bufs=2, space="PSUM") as pss, \
             tc.tile_pool(name="psa", bufs=1, space="PSUM") as psa, \
             tc.tile_pool(name="pso", bufs=2, space="PSUM") as pso:
            # our rank's query-column window within each gathered chunk
            col0 = (nc.sync.partition_id() % CPB) * 128
            cc_pending = []

            def flush_cc():
                while cc_pending:
                    q = cc_pending.pop(0)
                    nc.gpsimd.collective_compute(
                        "AllGather", AluOpType.bypass, replica_groups=GROUPS,
                        ins=[ag_in[q].opt()], outs=[ag_out[q].opt()])

            def oproj(qc):
                # logically delay past all attention so the scheduler cannot
                # hoist these ahead in the queues (they wait on a collective;
                # hoisting head-of-line-blocks the tensor queue behind it)
                ctx = tc.tile_wait_until(1.0 + 0.001 * qc)
                ctx.__enter__()
                gath = gathp.tile([128, 8 * 128], F16, tag="gath",
                                  name="gath")
                # all 16 heads for our rank's 128 queries in one DMA
                nc.sync.dma_start(
                    gath[:].rearrange("p (b c) -> p b c", b=8),
                    ag_out[qc][:].rearrange(
                        "(b p) q -> p b q", b=8)[:, :, bass.ds(col0, 128)])
                t_out = outp.tile([128, 1024], F32, tag="tout", name="t_out")
                for nn in range(2):
                    ps = pso.tile([128, 512], F32, tag="pso", name="ps")
                    for hb in range(HC):
                        nc.tensor.matmul(
                            ps[:], gath[:, hb * 128:(hb + 1) * 128],
                            wo_t[:, hb * HID + nn * 512:
                                 hb * HID + (nn + 1) * 512],
                            start=(hb == 0), stop=(hb == HC - 1))
                    nc.vector.tensor_copy(
                        t_out[:, nn * 512:(nn + 1) * 512], ps[:])
                nc.sync.dma_start(out[qc * 128:(qc + 1) * 128, :], t_out[:])
                ctx.__exit__(None, None, None)

            for qc in range(QC):
                n_kc = 4 * (qc + 1) if causal else KC
                for p in range(2):
                    ps_a = [psa.tile([65, 512], F32, tag=f"psa{h}",
                                     name=f"psa{h}") for h in range(2)]
                    for kc in range(n_kc):
                        # on diagonal blocks only queries >= key block are
                        # live: restrict everything to q in [q0w, 512)
                        q0w = max(0, kc - 4 * qc) * 128 if causal else 0
                        pse = pss.tile([128, 1024], F32, tag="pse")
                        for half in range(2):
                            hb = half * 64
                            nc.tensor.matmul(
                                pse[:, half * 512 + q0w:(half + 1) * 512],
                                kt[p][hb:hb + 64, kc * 128:(kc + 1) * 128],
                                qt[p][hb:hb + 64,
                                      qc * 512 + q0w:(qc + 1) * 512],
                                start=True, stop=True)
                        tex = expp.tile([128, 1024], F16, tag="tex")
                        if q0w == 0:
                            nc.scalar.activation(tex[:], pse[:], AF.Exp)
                        else:
                            for half in range(2):
                                nc.scalar.activation(
                                    tex[:, half * 512 + q0w:(half + 1) * 512],
                                    pse[:, half * 512 + q0w:(half + 1) * 512],
                                    AF.Exp)
                        if causal and kc >= 4 * qc:
                            tem = expp.tile([128, 1024], F16, tag="tem")
                            em2 = em_sb[qc][:, kc - 4 * qc, :]
                            for half in range(2):
                                nc.vector.tensor_tensor(
                                    tem[:, half * 512 + q0w:(half + 1) * 512],
                                    tex[:, half * 512 + q0w:(half + 1) * 512],
                                    em2[half * 512 + q0w:(half + 1) * 512]
                                    if False else
                                    em2[:, half * 512 + q0w:
                                        (half + 1) * 512],
                                    AluOpType.mult)
                        elif not causal:
                            tem = expp.tile([128, 1024], F16, tag="tem")
                            for half in range(2):
                                nc.vector.tensor_tensor(
                                    tem[:, half * 512:(half + 1) * 512],
                                    tex[:, half * 512:(half + 1) * 512],
                                    em_sb[qc][:, kc, :], AluOpType.mult)
                        else:
                            tem = tex
                        for half in range(2):
                            h = 2 * p + half
                            nc.tensor.matmul(
                                ps_a[half][:, q0w:512],
                                v_sb[kc][:, h * 65:h * 65 + 65],
                                tem[:, half * 512 + q0w:(half + 1) * 512],
                                start=(kc == 0), stop=(kc == n_kc - 1))
                    for half in range(2):
                        hb = half * 64
                        # den lives at PSUM partition 64; hop it to partition
                        # 0 (32-aligned cross-partition copy is legal), recip
                        # there, then broadcast (which always reads part. 0)
                        rec0 = nrm.tile([1, 512], F32, tag="rec0")
                        nc.vector.tensor_copy(rec0[:], ps_a[half][64:65, :])
                        rect = nrm.tile([1, 512], F32, tag="rect")
                        nc.vector.reciprocal_approx_fast(rect[:], rec0[:])
                        recb = nrm.tile([64, 512], F32, tag="recb")
                        nc.gpsimd.partition_broadcast(recb[:], rect[:])
                        with nc.allow_low_precision(reason="fp16 attn out"):
                            nc.vector.tensor_tensor(
                                acc2[p][hb:hb + 64, qc * 512:(qc + 1) * 512],
                                ps_a[half][0:64, :], recb[:], AluOpType.mult)
                # ship this chunk's normalized outputs: last chunk goes out
                # per pair (the pair-0 gather hides under pair-1 attention)
                for p in range(2):
                    nc.sync.dma_start(
                        ag_in[qc][p * 128:(p + 1) * 128, :],
                        acc2[p][:, qc * 512:(qc + 1) * 512])
                # defer the CC issue one chunk: the collective blocks the
                # gpsimd queue while it runs, so issue it only after the
                # NEXT chunk's broadcasts are already enqueued
                flush_cc()
                cc_pending.append(qc)
                if qc == QC - 1:
                    flush_cc()
                # o_proj for the PREVIOUS chunk (its exchange overlapped this
                # chunk's attention) - keeps the tensor queue from stalling
                if qc > 1:
                    oproj(qc - 2)
            oproj(QC - 2)
            oproj(QC - 1)

    nc.compile()
    return nc


_NC_CACHE = {}


def _get_program(causal: bool = True):
    if causal not in _NC_CACHE:
        _NC_CACHE[causal] = build_program(causal)
    return _NC_CACHE[causal]


def _detect_causal(attention_mask):
    """True if everything at or above the 512-block diagonal's upper edge is
    masked off hard enough that exp(mask) == 0 for our purposes."""
    m = np.asarray(attention_mask)  # [B, 1, S(q), S(k)]
    for qc in range(QC):
        k0 = (qc + 1) * 512
        if k0 >= S:
            continue
        blk = m[:, 0, qc * 512:(qc + 1) * 512, k0:]
        if not np.all(blk < -30.0):
            return False
    return True


def make_in_maps(hidden_states, attention_mask, position_ids, cos, sin,
                 Wq, Wk, Wv, Wo, causal):
    import ml_dtypes
    bf16 = ml_dtypes.bfloat16
    hidden_states = np.asarray(hidden_states, np.float32)
    attention_mask = np.asarray(attention_mask, np.float32)
    position_ids = np.asarray(position_ids)
    cos = np.asarray(cos, np.float32)
    sin = np.asarray(sin, np.float32)
    wq_f = np.asarray(Wq, np.float32) * SCALE
    wk_f = np.asarray(Wk, np.float32)
    wv_f = np.asarray(Wv, np.float32)
    wo_ = np.ascontiguousarray(np.asarray(Wo, np.float32)).astype(np.float16)

    in_maps = []
    for b in range(B):
        hsT_b = np.ascontiguousarray(hidden_states[b].T).astype(bf16)
        cos_b = cos[position_ids[b]]  # [S, HD]
        sin_b = sin[position_ids[b]]
        cosT = np.tile(cos_b.T, (2, 1)).astype(bf16)  # [128, S]
        # signed sin: the device swaps k's 32-row halves (rotate-half), so the
        # table stays index-aligned and only carries rotate-half's sign
        sin64 = sin_b.T  # [64, S]
        sh = np.empty_like(sin64)
        sh[0:32] = -sin64[0:32]
        sh[32:64] = sin64[32:64]
        sinT = np.tile(sh, (2, 1)).astype(bf16)  # [128, S]
        mask_b = attention_mask[b, 0]  # [S(q), S(k)]
        if causal:
            em = np.empty((S, 1024), np.float16)
            for qc in range(QC):
                blk = mask_b[qc * 512:(qc + 1) * 512,
                             qc * 512:(qc + 1) * 512].T  # [k, q]
                e = np.exp(blk).astype(np.float16)
                em[qc * 512:(qc + 1) * 512, 0:512] = e
                em[qc * 512:(qc + 1) * 512, 512:1024] = e
        else:
            em = np.exp(mask_b.T).astype(np.float16)  # [k, q]
        for g in range(CPB):
            c0 = g * HPC * HD
            in_maps.append({
                "hsT": hsT_b, "cosk": cosT, "sink": sinT, "emask": em,
                "wq": np.ascontiguousarray(wq_f[:, c0:c0 + HPC * HD]).astype(bf16),
                "wk": np.ascontiguousarray(wk_f[:, c0:c0 + HPC * HD]).astype(bf16),
                "wv": np.ascontiguousarray(wv_f[:, c0:c0 + HPC * HD]).astype(bf16),
                "wo": wo_,
            })
    return in_maps


def run(inputs: dict, trace: bool = False):
    causal = _detect_causal(inputs["attention_mask"])
    nc = _get_program(causal)
    in_maps = make_in_maps(**inputs, causal=causal)
    res = run_bass_kernel_spmd(nc, in_maps, list(range(N_CORES)), trace=trace)
    out = np.empty((B, S, HID), np.float32)
    for c in range(N_CORES):
        b, r = c // CPB, c % CPB
        for qc in range(QC):
            q0 = qc * 512 + r * 128
            out[b, q0:q0 + 128, :] = res.results[c]["out"][qc * 128:(qc + 1) * 128]
    return out, res


def kernel(**inputs) -> np.ndarray:
    out, _ = run(inputs, trace=False)
    return out


# revision 35
# speedup vs baseline: 1.0186x; 1.0186x over previous
"""Multi-head attention (RoPE + causal-mask softmax) on 8 TRN2 NeuronCores.

Sharding: batch x head-group (2 batches x 4 groups of 4 heads). Each core
computes q/k/v projections for its 4 heads over the full sequence and
attention for all 2048 queries. Per 512-query chunk, an AllGather over the
4 cores of the batch exchanges normalized attention outputs; each core then
reads its rank's 128-query column window (runtime-offset DMA) and runs the
full 16-head o_proj locally, so no partial-sum reduction is needed.

Head-sharding keeps the program SPMD-uniform while letting the causal
structure skip score blocks above the block diagonal (every core sees the
same query/key trapezoid). kernel() inspects the mask at runtime: if it is
(effectively) causal it builds the trapezoid program, otherwise a full-mask
fallback program.
"""

from contextlib import ExitStack

import numpy as np

import concourse.bass as bass
import concourse.tile as tile
from concourse import bacc, mybir
from concourse.alu_op_type import AluOpType
from concourse.bass_utils import run_bass_kernel_spmd

AF = mybir.ActivationFunctionType
F32 = mybir.dt.float32
F16 = mybir.dt.float16
BF16 = mybir.dt.bfloat16

B, S, HID, NH, HD = 2, 2048, 1024, 16, 64
SCALE = 1.0 / np.sqrt(HD)
N_CORES = 8
HPC = 4            # heads per core
CPB = 4            # cores per batch
HC = HID // 128    # hidden chunks (8)
QC = S // 512      # query chunks of 512 (4)
KC = S // 128      # key chunks of 128 (16)
GROUPS = [[0, 1, 2, 3], [4, 5, 6, 7]]


def build_program(causal: bool):
    nc = bacc.Bacc("TRN2", target_bir_lowering=False, debug=False,
                   num_devices=N_CORES)

    hsT = nc.dram_tensor("hsT", [HID, S], BF16, kind="ExternalInput").ap()
    cosk = nc.dram_tensor("cosk", [128, S], BF16, kind="ExternalInput").ap()
    sink = nc.dram_tensor("sink", [128, S], BF16, kind="ExternalInput").ap()
    # causal: exp(mask) diag blocks, [keys 512 per qc stacked, q 512 x2 dup]
    # general: exp(mask) full, [keys S, q S]
    em_cols = 1024 if causal else S
    emask = nc.dram_tensor("emask", [S, em_cols], F16, kind="ExternalInput").ap()
    wq = nc.dram_tensor("wq", [HID, HPC * HD], BF16, kind="ExternalInput").ap()
    wk = nc.dram_tensor("wk", [HID, HPC * HD], BF16, kind="ExternalInput").ap()
    wv = nc.dram_tensor("wv", [HID, HPC * HD], BF16, kind="ExternalInput").ap()
    wo = nc.dram_tensor("wo", [HID, HID], F16, kind="ExternalInput").ap()
    out = nc.dram_tensor("out", [512, HID], F32, kind="ExternalOutput").ap()

    with tile.TileContext(nc) as tc, ExitStack() as top:
        res = top.enter_context(tc.tile_pool(name="res", bufs=1))
        dram = top.enter_context(tc.tile_pool(name="dram", bufs=1, space="DRAM"))

        # AllGather exchange buffers, one per query chunk: each core
        # contributes its normalized [2 pairs x 128, 512 q] block; after the
        # gather, rows [g*256 + p*128] hold peer g's pair-p heads and every
        # core reads its own 128-query column window (rank-dynamic offset).
        ag_in = [dram.tile([256, 512], F16, tag=f"ai{qc}", name=f"ai{qc}")
                 for qc in range(QC)]
        ag_out = [dram.tile([CPB * 256, 512], F16, tag=f"ao{qc}",
                            name=f"ao{qc}") for qc in range(QC)]
        agl_in = [dram.tile([128, 512], F16, tag=f"ali{p}", name=f"ali{p}")
                  for p in range(2)]
        agl_out = [dram.tile([CPB * 128, 512], F16, tag=f"alo{p}",
                             name=f"alo{p}") for p in range(2)]

        # ---- resident tiles, batched multi-dim DMA loads --------------------
        # hsT as one [128, HC*S] tile; per-seq-chunk loads spread across the
        # three DMA-capable queues so the K projection can start early.
        hsT_t = res.tile([128, HC * S], BF16, tag="hsT")
        hs3 = hsT_t[:].rearrange("p (c s) -> p c s", c=HC)
        wk_t = res.tile([128, HC * 256], BF16, tag="wk")
        nc.gpsimd.dma_start(wk_t[:].rearrange("p (c n) -> p c n", c=HC),
                            wk[:].rearrange("(c p) n -> p c n", c=HC))
        # first seq chunk split across two queues so K proj starts early
        nc.sync.dma_start(
            hs3[:, 0:4, 0:512],
            hsT[0:512, 0:512].rearrange("(c p) q -> p c q", c=4))
        nc.scalar.dma_start(
            hs3[:, 4:8, 0:512],
            hsT[512:1024, 0:512].rearrange("(c p) q -> p c q", c=4))
        for sc, eng in ((1, nc.sync), (2, nc.scalar), (3, nc.sync)):
            eng.dma_start(
                hs3[:, :, sc * 512:(sc + 1) * 512],
                hsT[:, sc * 512:(sc + 1) * 512].rearrange(
                    "(c p) q -> p c q", c=HC))

        def hsv(hc):
            return hsT_t[:, hc * S:(hc + 1) * S]

        wq_t = res.tile([128, HC * 256], BF16, tag="wq")
        nc.gpsimd.dma_start(wq_t[:].rearrange("p (c n) -> p c n", c=HC),
                            wq[:].rearrange("(c p) n -> p c n", c=HC))
        wv_t = res.tile([128, HC * 256], BF16, tag="wv")
        nc.gpsimd.dma_start(wv_t[:].rearrange("p (c n) -> p c n", c=HC),
                            wv[:].rearrange("(c p) n -> p c n", c=HC))
        cos_sb = res.tile([128, S], BF16, tag="cos")
        nc.scalar.dma_start(cos_sb[:], cosk[:])
        sin_sb = res.tile([128, S], BF16, tag="sin")
        nc.scalar.dma_start(sin_sb[:], sink[:])
        # K/Q pair-packed [head dims: pair head A 0:64, head B 64:128]
        kt = [res.tile([128, S], BF16, tag=f"kt{p}", name=f"kt{p}")
              for p in range(2)]
        qt = [res.tile([128, S], BF16, tag=f"qt{p}", name=f"qt{p}")
              for p in range(2)]
        # V augmented: per key-chunk, 4 heads x (64 cols + ones col)
        v_sb = [res.tile([128, HPC * 65], F16, tag=f"v{kc}", name=f"v{kc}")
                for kc in range(KC)]
        ones4 = res.tile([128, HPC], F16, tag="ones4")
        nc.gpsimd.memset(ones4[:], 1.0)
        # normalized attention output, pair-packed
        acc2 = [res.tile([128, S], F16, tag=f"acc2_{p}", name=f"acc2_{p}")
                for p in range(2)]

        # ---- K/Q projection + RoPE -----------------------------------------
        # Full-width rope per pair: project all 4 seq chunks into kraw, swap
        # the 32-row rotate-half blocks with 4 wide SBUF->SBUF DMAs, then 3
        # full-width vector ops. sin_sb carries rotate-half's sign.
        with tc.tile_pool(name="rope", bufs=2) as rope, \
             tc.tile_pool(name="psk", bufs=2, space="PSUM") as psk:
            for w_t, dst_l in ((wk_t, kt), (wq_t, qt)):
                for p in range(2):
                    with nc.allow_low_precision(reason="bf16 rope"):
                        kraw = rope.tile([128, S], BF16, tag="kraw")
                        for sc in range(4):
                            ps = psk.tile([128, 512], F32, tag="psk")
                            for hc in range(HC):
                                nc.tensor.matmul(
                                    ps[:],
                                    w_t[:, hc * 256 + p * 128:
                                        hc * 256 + (p + 1) * 128],
                                    hsv(hc)[:, sc * 512:(sc + 1) * 512],
                                    start=(hc == 0), stop=(hc == HC - 1))
                            nc.vector.tensor_copy(
                                kraw[:, sc * 512:(sc + 1) * 512], ps[:])
                        ksw = rope.tile([128, S], BF16, tag="ksw")
                        for hb in (0, 64):
                            nc.gpsimd.dma_start(ksw[hb:hb + 32, :],
                                                kraw[hb + 32:hb + 64, :])
                            nc.gpsimd.dma_start(ksw[hb + 32:hb + 64, :],
                                                kraw[hb:hb + 32, :])
                        t1 = rope.tile([128, S], BF16, tag="t1")
                        nc.vector.tensor_tensor(t1[:], kraw[:], cos_sb[:],
                                                AluOpType.mult)
                        t2 = rope.tile([128, S], BF16, tag="t2")
                        nc.vector.tensor_tensor(t2[:], ksw[:], sin_sb[:],
                                                AluOpType.mult)
                        nc.vector.tensor_tensor(dst_l[p][:], t1[:], t2[:],
                                                AluOpType.add)

        # ---- mid loads: wo + exp(mask), transfer during the rope window -----
        wo_t = res.tile([128, HC * HID], F16, tag="wo")
        nc.scalar.dma_start(wo_t[:].rearrange("p (c n) -> p c n", c=HC),
                            wo[:].rearrange("(c p) n -> p c n", c=HC))
        em_sb = []
        nkc_em = 4 if causal else KC
        em_w = 1024 if causal else 512
        for qc in range(QC):
            t = res.tile([128, nkc_em * em_w], F16, tag=f"em{qc}",
                         name=f"em{qc}")
            esrc = (emask[qc * 512:(qc + 1) * 512, :] if causal
                    else emask[:, qc * 512:(qc + 1) * 512])
            nc.scalar.dma_start(
                t[:].rearrange("p (c q) -> p c q", c=nkc_em),
                esrc.rearrange("(c p) q -> p c q", c=nkc_em))
            em_sb.append(t[:].rearrange("p (c q) -> p c q", c=nkc_em))

        # ---- V projection ---------------------------------------------------
        with tc.tile_pool(name="psv", bufs=2, space="PSUM") as psv:
            for kc in range(KC):
                ps = psv.tile([128, HPC * HD], F32, tag="psv")
                for hc in range(HC):
                    nc.tensor.matmul(
                        ps[:], hsv(hc)[:, kc * 128:(kc + 1) * 128],
                        wv_t[:, hc * 256:(hc + 1) * 256],
                        start=(hc == 0), stop=(hc == HC - 1))
                v3 = v_sb[kc][:].rearrange("p (h c) -> p h c", h=HPC)
                ps3 = ps[:].rearrange("p (h c) -> p h c", h=HPC)
                with nc.allow_low_precision(reason="fp16 v"):
                    nc.vector.tensor_copy(v3[:, :, 0:64], ps3[:])
                nc.gpsimd.tensor_copy(v3[:, :, 64], ones4[:])

        # ---- attention + exchange + o_proj, per query chunk -----------------
        with tc.tile_pool(name="expp", bufs=4) as expp, \
             tc.tile_pool(name="nrm", bufs=2) as nrm, \
             tc.tile_pool(name="gath", bufs=2) as gathp, \
             tc.tile_pool(name="outp", bufs=2) as outp, \
             tc.tile_pool(name="pss", bufs=2, space="PSUM") as pss, \
             tc.tile_pool(name="psa", bufs=1, space="PSUM") as psa, \
             tc.tile_pool(name="pso", bufs=2, space="PSUM") as pso:
            # our rank's query-column window within each gathered chunk
            col0 = (nc.sync.partition_id() % CPB) * 128
            cc_pending = []

            def flush_cc():
                while cc_pending:
                    q = cc_pending.pop(0)
                    nc.gpsimd.collective_compute(
                        "AllGather", AluOpType.bypass, replica_groups=GROUPS,
                        ins=[ag_in[q].opt()], outs=[ag_out[q].opt()])

            def oproj(qc):
                # logically delay past all attention so the scheduler cannot
                # hoist these ahead in the queues (they wait on a collective;
                # hoisting head-of-line-blocks the tensor queue behind it)
                ctx = tc.tile_wait_until(1.0 + 0.001 * qc)
                ctx.__enter__()
                gath = gathp.tile([128, 8 * 128], F16, tag="gath",
                                  name="gath")
                if qc < QC - 1:
                    # all 16 heads for our rank's 128 queries in one DMA
                    nc.sync.dma_start(
                        gath[:].rearrange("p (b c) -> p b c", b=8),
                        ag_out[qc][:].rearrange(
                            "(b p) q -> p b q",
                            b=8)[:, :, bass.ds(col0, 128)])
                else:
                    for p in range(2):
                        nc.sync.dma_start(
                            gath[:].rearrange("p (g t c) -> p t g c", g=CPB,
                                              t=2)[:, p, :, :],
                            agl_out[p][:].rearrange(
                                "(g x) q -> x g q",
                                g=CPB)[:, :, bass.ds(col0, 128)])
                t_out = outp.tile([128, 1024], F32, tag="tout", name="t_out")
                for nn in range(2):
                    ps = pso.tile([128, 512], F32, tag="pso", name="ps")
                    for hb in range(HC):
                        nc.tensor.matmul(
                            ps[:], gath[:, hb * 128:(hb + 1) * 128],
                            wo_t[:, hb * HID + nn * 512:
                                 hb * HID + (nn + 1) * 512],
                            start=(hb == 0), stop=(hb == HC - 1))
                    nc.vector.tensor_copy(
                        t_out[:, nn * 512:(nn + 1) * 512], ps[:])
                nc.sync.dma_start(out[qc * 128:(qc + 1) * 128, :], t_out[:])
                ctx.__exit__(None, None, None)

            for qc in range(QC):
                n_kc = 4 * (qc + 1) if causal else KC
                for p in range(2):
                    ps_a = [psa.tile([65, 512], F32, tag=f"psa{h}",
                                     name=f"psa{h}") for h in range(2)]
                    for kc in range(n_kc):
                        # on diagonal blocks only queries >= key block are
                        # live: restrict everything to q in [q0w, 512)
                        q0w = max(0, kc - 4 * qc) * 128 if causal else 0
                        pse = pss.tile([128, 1024], F32, tag="pse")
                        for half in range(2):
                            hb = half * 64
                            nc.tensor.matmul(
                                pse[:, half * 512 + q0w:(half + 1) * 512],
                                kt[p][hb:hb + 64, kc * 128:(kc + 1) * 128],
                                qt[p][hb:hb + 64,
                                      qc * 512 + q0w:(qc + 1) * 512],
                                start=True, stop=True)
                        tex = expp.tile([128, 1024], F16, tag="tex")
                        if q0w == 0:
                            nc.scalar.activation(tex[:], pse[:], AF.Exp)
                        else:
                            for half in range(2):
                                nc.scalar.activation(
                                    tex[:, half * 512 + q0w:(half + 1) * 512],
                                    pse[:, half * 512 + q0w:(half + 1) * 512],
                                    AF.Exp)
                        if causal and kc >= 4 * qc:
                            tem = expp.tile([128, 1024], F16, tag="tem")
                            em2 = em_sb[qc][:, kc - 4 * qc, :]
                            for half in range(2):
                                nc.vector.tensor_tensor(
                                    tem[:, half * 512 + q0w:(half + 1) * 512],
                                    tex[:, half * 512 + q0w:(half + 1) * 512],
                                    em2[half * 512 + q0w:(half + 1) * 512]
                                    if False else
                                    em2[:, half * 512 + q0w:
                                        (half + 1) * 512],
                                    AluOpType.mult)
                        elif not causal:
                            tem = expp.tile([128, 1024], F16, tag="tem")
                            for half in range(2):
                                nc.vector.tensor_tensor(
                                    tem[:, half * 512:(half + 1) * 512],
                                    tex[:, half * 512:(half + 1) * 512],
                                    em_sb[qc][:, kc, :], AluOpType.mult)
                        else:
                            tem = tex
                        for half in range(2):
                            h = 2 * p + half
                            nc.tensor.matmul(
                                ps_a[half][:, q0w:512],
                                v_sb[kc][:, h * 65:h * 65 + 65],
                                tem[:, half * 512 + q0w:(half + 1) * 512],
                                start=(kc == 0), stop=(kc == n_kc - 1))
                    for half in range(2):
                        hb = half * 64
                        # den lives at PSUM partition 64; hop it to partition
                        # 0 (32-aligned cross-partition copy is legal), recip
                        # there, then broadcast (which always reads part. 0)
                        rec0 = nrm.tile([1, 512], F32, tag="rec0")
                        nc.vector.tensor_copy(rec0[:], ps_a[half][64:65, :])
                        rect = nrm.tile([1, 512], F32, tag="rect")
                        nc.vector.reciprocal_approx_fast(rect[:], rec0[:])
                        recb = nrm.tile([64, 512], F32, tag="recb")
                        nc.gpsimd.partition_broadcast(recb[:], rect[:])
                        with nc.allow_low_precision(reason="fp16 attn out"):
                            nc.vector.tensor_tensor(
                                acc2[p][hb:hb + 64, qc * 512:(qc + 1) * 512],
                                ps_a[half][0:64, :], recb[:], AluOpType.mult)
                    if qc == QC - 1:
                        nc.sync.dma_start(
                            agl_in[p][:], acc2[p][:, qc * 512:(qc + 1) * 512])
                        flush_cc()
                        nc.gpsimd.collective_compute(
                            "AllGather", AluOpType.bypass,
                            replica_groups=GROUPS,
                            ins=[agl_in[p].opt()], outs=[agl_out[p].opt()])
                # ship this chunk's normalized outputs: last chunk goes out
                # per pair (the pair-0 gather hides under pair-1 attention)
                if qc < QC - 1:
                    for p in range(2):
                        nc.sync.dma_start(
                            ag_in[qc][p * 128:(p + 1) * 128, :],
                            acc2[p][:, qc * 512:(qc + 1) * 512])
                    # defer the CC issue one chunk: the collective blocks
                    # the gpsimd queue while it runs, so issue it only after
                    # the NEXT chunk's broadcasts are already enqueued
                    flush_cc()
                    cc_pending.append(qc)
                # o_proj for the PREVIOUS chunk (its exchange overlapped this
                # chunk's attention) - keeps the tensor queue from stalling
                if qc > 1:
                    oproj(qc - 2)
            oproj(QC - 2)
            oproj(QC - 1)

    nc.compile()
    return nc


_NC_CACHE = {}


def _get_program(causal: bool = True):
    if causal not in _NC_CACHE:
        _NC_CACHE[causal] = build_program(causal)
    return _NC_CACHE[causal]


def _detect_causal(attention_mask):
    """True if everything at or above the 512-block diagonal's upper edge is
    masked off hard enough that exp(mask) == 0 for our purposes."""
    m = np.asarray(attention_mask)  # [B, 1, S(q), S(k)]
    for qc in range(QC):
        k0 = (qc + 1) * 512
        if k0 >= S:
            continue
        blk = m[:, 0, qc * 512:(qc + 1) * 512, k0:]
        if not np.all(blk < -30.0):
            return False
    return True


def make_in_maps(hidden_states, attention_mask, position_ids, cos, sin,
                 Wq, Wk, Wv, Wo, causal):
    import ml_dtypes
    bf16 = ml_dtypes.bfloat16
    hidden_states = np.asarray(hidden_states, np.float32)
    attention_mask = np.asarray(attention_mask, np.float32)
    position_ids = np.asarray(position_ids)
    cos = np.asarray(cos, np.float32)
    sin = np.asarray(sin, np.float32)
    wq_f = np.asarray(Wq, np.float32) * SCALE
    wk_f = np.asarray(Wk, np.float32)
    wv_f = np.asarray(Wv, np.float32)
    wo_ = np.ascontiguousarray(np.asarray(Wo, np.float32)).astype(np.float16)

    in_maps = []
    for b in range(B):
        hsT_b = np.ascontiguousarray(hidden_states[b].T).astype(bf16)
        cos_b = cos[position_ids[b]]  # [S, HD]
        sin_b = sin[position_ids[b]]
        cosT = np.tile(cos_b.T, (2, 1)).astype(bf16)  # [128, S]
        # signed sin: the device swaps k's 32-row halves (rotate-half), so the
        # table stays index-aligned and only carries rotate-half's sign
        sin64 = sin_b.T  # [64, S]
        sh = np.empty_like(sin64)
        sh[0:32] = -sin64[0:32]
        sh[32:64] = sin64[32:64]
        sinT = np.tile(sh, (2, 1)).astype(bf16)  # [128, S]
        mask_b = attention_mask[b, 0]  # [S(q), S(k)]
        if causal:
            em = np.empty((S, 1024), np.float16)
            for qc in range(QC):
                blk = mask_b[qc * 512:(qc + 1) * 512,
                             qc * 512:(qc + 1) * 512].T  # [k, q]
                e = np.exp(blk).astype(np.float16)
                em[qc * 512:(qc + 1) * 512, 0:512] = e
                em[qc * 512:(qc + 1) * 512, 512:1024] = e
        else:
            em = np.exp(mask_b.T).astype(np.float16)  # [k, q]
        for g in range(CPB):
            c0 = g * HPC * HD
            in_maps.append({
                "hsT": hsT_b, "cosk": cosT, "sink": sinT, "emask": em,
                "wq": np.ascontiguousarray(wq_f[:, c0:c0 + HPC * HD]).astype(bf16),
                "wk": np.ascontiguousarray(wk_f[:, c0:c0 + HPC * HD]).astype(bf16),
                "wv": np.ascontiguousarray(wv_f[:, c0:c0 + HPC * HD]).astype(bf16),
                "wo": wo_,
            })
    return in_maps


def run(inputs: dict, trace: bool = False):
    causal = _detect_causal(inputs["attention_mask"])
    nc = _get_program(causal)
    in_maps = make_in_maps(**inputs, causal=causal)
    res = run_bass_kernel_spmd(nc, in_maps, list(range(N_CORES)), trace=trace)
    out = np.empty((B, S, HID), np.float32)
    for c in range(N_CORES):
        b, r = c // CPB, c % CPB
        for qc in range(QC):
            q0 = qc * 512 + r * 128
            out[b, q0:q0 + 128, :] = res.results[c]["out"][qc * 128:(qc + 1) * 128]
    return out, res


def kernel(**inputs) -> np.ndarray:
    out, _ = run(inputs, trace=False)
    return out


# revision 36
# speedup vs baseline: 1.0326x; 1.0137x over previous
"""Multi-head attention (RoPE + causal-mask softmax) on 8 TRN2 NeuronCores.

Sharding: batch x head-group (2 batches x 4 groups of 4 heads). Each core
computes q/k/v projections for its 4 heads over the full sequence and
attention for all 2048 queries. Per 512-query chunk, an AllGather over the
4 cores of the batch exchanges normalized attention outputs; each core then
reads its rank's 128-query column window (runtime-offset DMA) and runs the
full 16-head o_proj locally, so no partial-sum reduction is needed.

Head-sharding keeps the program SPMD-uniform while letting the causal
structure skip score blocks above the block diagonal (every core sees the
same query/key trapezoid). kernel() inspects the mask at runtime: if it is
(effectively) causal it builds the trapezoid program, otherwise a full-mask
fallback program.
"""

from contextlib import ExitStack

import numpy as np

import concourse.bass as bass
import concourse.tile as tile
from concourse import bacc, mybir
from concourse.alu_op_type import AluOpType
from concourse.bass_utils import run_bass_kernel_spmd

AF = mybir.ActivationFunctionType
F32 = mybir.dt.float32
F16 = mybir.dt.float16
BF16 = mybir.dt.bfloat16

B, S, HID, NH, HD = 2, 2048, 1024, 16, 64
SCALE = 1.0 / np.sqrt(HD)
N_CORES = 8
HPC = 4            # heads per core
CPB = 4            # cores per batch
HC = HID // 128    # hidden chunks (8)
QC = S // 512      # query chunks of 512 (4)
KC = S // 128      # key chunks of 128 (16)
GROUPS = [[0, 1, 2, 3], [4, 5, 6, 7]]


def build_program(causal: bool):
    nc = bacc.Bacc("TRN2", target_bir_lowering=False, debug=False,
                   num_devices=N_CORES)

    hsT = nc.dram_tensor("hsT", [HID, S], BF16, kind="ExternalInput").ap()
    cosk = nc.dram_tensor("cosk", [128, S], BF16, kind="ExternalInput").ap()
    sink = nc.dram_tensor("sink", [128, S], BF16, kind="ExternalInput").ap()
    # causal: exp(mask) diag blocks, [keys 512 per qc stacked, q 512 x2 dup]
    # general: exp(mask) full, [keys S, q S]
    em_cols = 1024 if causal else S
    emask = nc.dram_tensor("emask", [S, em_cols], F16, kind="ExternalInput").ap()
    wq = nc.dram_tensor("wq", [HID, HPC * HD], BF16, kind="ExternalInput").ap()
    wk = nc.dram_tensor("wk", [HID, HPC * HD], BF16, kind="ExternalInput").ap()
    wv = nc.dram_tensor("wv", [HID, HPC * HD], BF16, kind="ExternalInput").ap()
    wo = nc.dram_tensor("wo", [HID, HID], F16, kind="ExternalInput").ap()
    out = nc.dram_tensor("out", [512, HID], F32, kind="ExternalOutput").ap()

    with tile.TileContext(nc) as tc, ExitStack() as top:
        res = top.enter_context(tc.tile_pool(name="res", bufs=1))
        dram = top.enter_context(tc.tile_pool(name="dram", bufs=1, space="DRAM"))

        # AllGather exchange buffers, one per query chunk: each core
        # contributes its normalized [2 pairs x 128, 512 q] block; after the
        # gather, rows [g*256 + p*128] hold peer g's pair-p heads and every
        # core reads its own 128-query column window (rank-dynamic offset).
        ag_in = [dram.tile([256, 512], F16, tag=f"ai{qc}", name=f"ai{qc}")
                 for qc in range(QC)]
        ag_out = [dram.tile([CPB * 256, 512], F16, tag=f"ao{qc}",
                            name=f"ao{qc}") for qc in range(QC)]
        agl_in = [dram.tile([128, 512], F16, tag=f"ali{p}", name=f"ali{p}")
                  for p in range(2)]
        agl_out = [dram.tile([CPB * 128, 512], F16, tag=f"alo{p}",
                             name=f"alo{p}") for p in range(2)]

        # ---- resident tiles, batched multi-dim DMA loads --------------------
        # hsT as one [128, HC*S] tile; per-seq-chunk loads spread across the
        # three DMA-capable queues so the K projection can start early.
        hsT_t = res.tile([128, HC * S], BF16, tag="hsT")
        hs3 = hsT_t[:].rearrange("p (c s) -> p c s", c=HC)
        wk_t = res.tile([128, HC * 256], BF16, tag="wk")
        nc.gpsimd.dma_start(wk_t[:].rearrange("p (c n) -> p c n", c=HC),
                            wk[:].rearrange("(c p) n -> p c n", c=HC))
        # first seq chunk split across queues so K proj starts early
        nc.sync.dma_start(
            hs3[:, 0:3, 0:512],
            hsT[0:384, 0:512].rearrange("(c p) q -> p c q", c=3))
        nc.scalar.dma_start(
            hs3[:, 3:6, 0:512],
            hsT[384:768, 0:512].rearrange("(c p) q -> p c q", c=3))
        nc.gpsimd.dma_start(
            hs3[:, 6:8, 0:512],
            hsT[768:1024, 0:512].rearrange("(c p) q -> p c q", c=2))
        for sc, eng in ((1, nc.sync), (2, nc.scalar), (3, nc.sync)):
            eng.dma_start(
                hs3[:, :, sc * 512:(sc + 1) * 512],
                hsT[:, sc * 512:(sc + 1) * 512].rearrange(
                    "(c p) q -> p c q", c=HC))

        def hsv(hc):
            return hsT_t[:, hc * S:(hc + 1) * S]

        wq_t = res.tile([128, HC * 256], BF16, tag="wq")
        nc.gpsimd.dma_start(wq_t[:].rearrange("p (c n) -> p c n", c=HC),
                            wq[:].rearrange("(c p) n -> p c n", c=HC))
        wv_t = res.tile([128, HC * 256], BF16, tag="wv")
        nc.gpsimd.dma_start(wv_t[:].rearrange("p (c n) -> p c n", c=HC),
                            wv[:].rearrange("(c p) n -> p c n", c=HC))
        cos_sb = res.tile([128, S], BF16, tag="cos")
        nc.scalar.dma_start(cos_sb[:], cosk[:])
        sin_sb = res.tile([128, S], BF16, tag="sin")
        nc.scalar.dma_start(sin_sb[:], sink[:])
        # K/Q pair-packed [head dims: pair head A 0:64, head B 64:128]
        kt = [res.tile([128, S], BF16, tag=f"kt{p}", name=f"kt{p}")
              for p in range(2)]
        qt = [res.tile([128, S], BF16, tag=f"qt{p}", name=f"qt{p}")
              for p in range(2)]
        # V augmented: per key-chunk, 4 heads x (64 cols + ones col)
        v_sb = [res.tile([128, HPC * 65], F16, tag=f"v{kc}", name=f"v{kc}")
                for kc in range(KC)]
        ones4 = res.tile([128, HPC], F16, tag="ones4")
        nc.gpsimd.memset(ones4[:], 1.0)
        # normalized attention output, pair-packed
        acc2 = [res.tile([128, S], F16, tag=f"acc2_{p}", name=f"acc2_{p}")
                for p in range(2)]

        # ---- K/Q projection + RoPE -----------------------------------------
        # Full-width rope per pair: project all 4 seq chunks into kraw, swap
        # the 32-row rotate-half blocks with 4 wide SBUF->SBUF DMAs, then 3
        # full-width vector ops. sin_sb carries rotate-half's sign.
        with tc.tile_pool(name="rope", bufs=2) as rope, \
             tc.tile_pool(name="psk", bufs=2, space="PSUM") as psk:
            for w_t, dst_l in ((wk_t, kt), (wq_t, qt)):
                for p in range(2):
                    with nc.allow_low_precision(reason="bf16 rope"):
                        kraw = rope.tile([128, S], BF16, tag="kraw")
                        for sc in range(4):
                            ps = psk.tile([128, 512], F32, tag="psk")
                            for hc in range(HC):
                                nc.tensor.matmul(
                                    ps[:],
                                    w_t[:, hc * 256 + p * 128:
                                        hc * 256 + (p + 1) * 128],
                                    hsv(hc)[:, sc * 512:(sc + 1) * 512],
                                    start=(hc == 0), stop=(hc == HC - 1))
                            nc.vector.tensor_copy(
                                kraw[:, sc * 512:(sc + 1) * 512], ps[:])
                        ksw = rope.tile([128, S], BF16, tag="ksw")
                        for hb in (0, 64):
                            nc.gpsimd.dma_start(ksw[hb:hb + 32, :],
                                                kraw[hb + 32:hb + 64, :])
                            nc.gpsimd.dma_start(ksw[hb + 32:hb + 64, :],
                                                kraw[hb:hb + 32, :])
                        t1 = rope.tile([128, S], BF16, tag="t1")
                        nc.vector.tensor_tensor(t1[:], kraw[:], cos_sb[:],
                                                AluOpType.mult)
                        t2 = rope.tile([128, S], BF16, tag="t2")
                        nc.vector.tensor_tensor(t2[:], ksw[:], sin_sb[:],
                                                AluOpType.mult)
                        nc.vector.tensor_tensor(dst_l[p][:], t1[:], t2[:],
                                                AluOpType.add)

        # ---- mid loads: wo + exp(mask), transfer during the rope window -----
        wo_t = res.tile([128, HC * HID], F16, tag="wo")
        nc.scalar.dma_start(wo_t[:].rearrange("p (c n) -> p c n", c=HC),
                            wo[:].rearrange("(c p) n -> p c n", c=HC))
        em_sb = []
        nkc_em = 4 if causal else KC
        em_w = 1024 if causal else 512
        for qc in range(QC):
            t = res.tile([128, nkc_em * em_w], F16, tag=f"em{qc}",
                         name=f"em{qc}")
            esrc = (emask[qc * 512:(qc + 1) * 512, :] if causal
                    else emask[:, qc * 512:(qc + 1) * 512])
            nc.scalar.dma_start(
                t[:].rearrange("p (c q) -> p c q", c=nkc_em),
                esrc.rearrange("(c p) q -> p c q", c=nkc_em))
            em_sb.append(t[:].rearrange("p (c q) -> p c q", c=nkc_em))

        # ---- V projection ---------------------------------------------------
        with tc.tile_pool(name="psv", bufs=2, space="PSUM") as psv:
            for kc in range(KC):
                ps = psv.tile([128, HPC * HD], F32, tag="psv")
                for hc in range(HC):
                    nc.tensor.matmul(
                        ps[:], hsv(hc)[:, kc * 128:(kc + 1) * 128],
                        wv_t[:, hc * 256:(hc + 1) * 256],
                        start=(hc == 0), stop=(hc == HC - 1))
                v3 = v_sb[kc][:].rearrange("p (h c) -> p h c", h=HPC)
                ps3 = ps[:].rearrange("p (h c) -> p h c", h=HPC)
                with nc.allow_low_precision(reason="fp16 v"):
                    nc.vector.tensor_copy(v3[:, :, 0:64], ps3[:])
                nc.gpsimd.tensor_copy(v3[:, :, 64], ones4[:])

        # ---- attention + exchange + o_proj, per query chunk -----------------
        with tc.tile_pool(name="expp", bufs=4) as expp, \
             tc.tile_pool(name="nrm", bufs=2) as nrm, \
             tc.tile_pool(name="gath", bufs=2) as gathp, \
             tc.tile_pool(name="outp", bufs=2) as outp, \
             tc.tile_pool(name="pss", bufs=3, space="PSUM") as pss, \
             tc.tile_pool(name="psa", bufs=1, space="PSUM") as psa:
            # our rank's query-column window within each gathered chunk
            col0 = (nc.sync.partition_id() % CPB) * 128
            cc_pending = []

            def flush_cc():
                while cc_pending:
                    q = cc_pending.pop(0)
                    nc.gpsimd.collective_compute(
                        "AllGather", AluOpType.bypass, replica_groups=GROUPS,
                        ins=[ag_in[q].opt()], outs=[ag_out[q].opt()])

            def oproj(qc):
                # logically delay past all attention so the scheduler cannot
                # hoist these ahead in the queues (they wait on a collective;
                # hoisting head-of-line-blocks the tensor queue behind it)
                ctx = tc.tile_wait_until(1.0 + 0.001 * qc)
                ctx.__enter__()
                gath = gathp.tile([128, 8 * 128], F16, tag="gath",
                                  name="gath")
                if qc < QC - 1:
                    # all 16 heads for our rank's 128 queries in one DMA
                    nc.sync.dma_start(
                        gath[:].rearrange("p (b c) -> p b c", b=8),
                        ag_out[qc][:].rearrange(
                            "(b p) q -> p b q",
                            b=8)[:, :, bass.ds(col0, 128)])
                else:
                    for p in range(2):
                        nc.sync.dma_start(
                            gath[:].rearrange("p (g t c) -> p t g c", g=CPB,
                                              t=2)[:, p, :, :],
                            agl_out[p][:].rearrange(
                                "(g x) q -> x g q",
                                g=CPB)[:, :, bass.ds(col0, 128)])
                t_out = outp.tile([128, 1024], F32, tag="tout", name="t_out")
                ps = pss.tile([128, 1024], F32, tag="pse", name="ps")
                # even head-blocks come from the pair-0 exchange, odd from
                # pair-1: accumulate evens first so only the last 4 matmuls
                # wait on the final collective
                hbs = [0, 2, 4, 6, 1, 3, 5, 7] if qc == QC - 1 else list(range(HC))
                for nn in range(2):
                    for i, hb in enumerate(hbs):
                        nc.tensor.matmul(
                            ps[:, nn * 512:(nn + 1) * 512],
                            gath[:, hb * 128:(hb + 1) * 128],
                            wo_t[:, hb * HID + nn * 512:
                                 hb * HID + (nn + 1) * 512],
                            start=(i == 0), stop=(i == HC - 1))
                    nc.vector.tensor_copy(
                        t_out[:, nn * 512:(nn + 1) * 512],
                        ps[:, nn * 512:(nn + 1) * 512])
                nc.sync.dma_start(out[qc * 128:(qc + 1) * 128, :], t_out[:])
                ctx.__exit__(None, None, None)

            for qc in range(QC):
                n_kc = 4 * (qc + 1) if causal else KC
                for p in range(2):
                    ps_a = [psa.tile([65, 512], F32, tag=f"psa{h}",
                                     name=f"psa{h}") for h in range(2)]
                    for kc in range(n_kc):
                        # on diagonal blocks only queries >= key block are
                        # live: restrict everything to q in [q0w, 512)
                        q0w = max(0, kc - 4 * qc) * 128 if causal else 0
                        pse = pss.tile([128, 1024], F32, tag="pse")
                        for half in range(2):
                            hb = half * 64
                            nc.tensor.matmul(
                                pse[:, half * 512 + q0w:(half + 1) * 512],
                                kt[p][hb:hb + 64, kc * 128:(kc + 1) * 128],
                                qt[p][hb:hb + 64,
                                      qc * 512 + q0w:(qc + 1) * 512],
                                start=True, stop=True)
                        tex = expp.tile([128, 1024], F16, tag="tex")
                        if q0w == 0:
                            nc.scalar.activation(tex[:], pse[:], AF.Exp)
                        else:
                            t3 = tex[:].rearrange("p (h q) -> p h q", h=2)
                            p3 = pse[:].rearrange("p (h q) -> p h q", h=2)
                            nc.scalar.activation(t3[:, :, q0w:],
                                                 p3[:, :, q0w:], AF.Exp)
                        if causal and kc >= 4 * qc:
                            tem = expp.tile([128, 1024], F16, tag="tem")
                            em2 = em_sb[qc][:, kc - 4 * qc, :]
                            for half in range(2):
                                nc.vector.tensor_tensor(
                                    tem[:, half * 512 + q0w:(half + 1) * 512],
                                    tex[:, half * 512 + q0w:(half + 1) * 512],
                                    em2[half * 512 + q0w:(half + 1) * 512]
                                    if False else
                                    em2[:, half * 512 + q0w:
                                        (half + 1) * 512],
                                    AluOpType.mult)
                        elif not causal:
                            tem = expp.tile([128, 1024], F16, tag="tem")
                            for half in range(2):
                                nc.vector.tensor_tensor(
                                    tem[:, half * 512:(half + 1) * 512],
                                    tex[:, half * 512:(half + 1) * 512],
                                    em_sb[qc][:, kc, :], AluOpType.mult)
                        else:
                            tem = tex
                        for half in range(2):
                            h = 2 * p + half
                            nc.tensor.matmul(
                                ps_a[half][:, q0w:512],
                                v_sb[kc][:, h * 65:h * 65 + 65],
                                tem[:, half * 512 + q0w:(half + 1) * 512],
                                start=(kc == 0), stop=(kc == n_kc - 1))
                    for half in range(2):
                        hb = half * 64
                        # den lives at PSUM partition 64; hop it to partition
                        # 0 (32-aligned cross-partition copy is legal), recip
                        # there, then broadcast (which always reads part. 0)
                        rec0 = nrm.tile([1, 512], F32, tag="rec0")
                        nc.vector.tensor_copy(rec0[:], ps_a[half][64:65, :])
                        rect = nrm.tile([1, 512], F32, tag="rect")
                        nc.vector.reciprocal_approx_fast(rect[:], rec0[:])
                        recb = nrm.tile([64, 512], F32, tag="recb")
                        nc.gpsimd.partition_broadcast(recb[:], rect[:])
                        with nc.allow_low_precision(reason="fp16 attn out"):
                            nc.vector.tensor_tensor(
                                acc2[p][hb:hb + 64, qc * 512:(qc + 1) * 512],
                                ps_a[half][0:64, :], recb[:], AluOpType.mult)
                    if qc == QC - 1:
                        nc.sync.dma_start(
                            agl_in[p][:], acc2[p][:, qc * 512:(qc + 1) * 512])
                        flush_cc()
                        nc.gpsimd.collective_compute(
                            "AllGather", AluOpType.bypass,
                            replica_groups=GROUPS,
                            ins=[agl_in[p].opt()], outs=[agl_out[p].opt()])
                # ship this chunk's normalized outputs: last chunk goes out
                # per pair (the pair-0 gather hides under pair-1 attention)
                if qc < QC - 1:
                    for p in range(2):
                        nc.sync.dma_start(
                            ag_in[qc][p * 128:(p + 1) * 128, :],
                            acc2[p][:, qc * 512:(qc + 1) * 512])
                    # defer the CC issue one chunk: the collective blocks
                    # the gpsimd queue while it runs, so issue it only after
                    # the NEXT chunk's broadcasts are already enqueued
                    flush_cc()
                    cc_pending.append(qc)
                # o_proj for the PREVIOUS chunk (its exchange overlapped this
                # chunk's attention) - keeps the tensor queue from stalling
                if qc > 1:
                    oproj(qc - 2)
            oproj(QC - 2)
            oproj(QC - 1)

    nc.compile()
    return nc


_NC_CACHE = {}


def _get_program(causal: bool = True):
    if causal not in _NC_CACHE:
        _NC_CACHE[causal] = build_program(causal)
    return _NC_CACHE[causal]


def _detect_causal(attention_mask):
    """True if everything at or above the 512-block diagonal's upper edge is
    masked off hard enough that exp(mask) == 0 for our purposes."""
    m = np.asarray(attention_mask)  # [B, 1, S(q), S(k)]
    for qc in range(QC):
        k0 = (qc + 1) * 512
        if k0 >= S:
            continue
        blk = m[:, 0, qc * 512:(qc + 1) * 512, k0:]
        if not np.all(blk < -30.0):
            return False
    return True


def make_in_maps(hidden_states, attention_mask, position_ids, cos, sin,
                 Wq, Wk, Wv, Wo, causal):
    import ml_dtypes
    bf16 = ml_dtypes.bfloat16
    hidden_states = np.asarray(hidden_states, np.float32)
    attention_mask = np.asarray(attention_mask, np.float32)
    position_ids = np.asarray(position_ids)
    cos = np.asarray(cos, np.float32)
    sin = np.asarray(sin, np.float32)
    wq_f = np.asarray(Wq, np.float32) * SCALE
    wk_f = np.asarray(Wk, np.float32)
    wv_f = np.asarray(Wv, np.float32)
    wo_ = np.ascontiguousarray(np.asarray(Wo, np.float32)).astype(np.float16)

    in_maps = []
    for b in range(B):
        hsT_b = np.ascontiguousarray(hidden_states[b].T).astype(bf16)
        cos_b = cos[position_ids[b]]  # [S, HD]
        sin_b = sin[position_ids[b]]
        cosT = np.tile(cos_b.T, (2, 1)).astype(bf16)  # [128, S]
        # signed sin: the device swaps k's 32-row halves (rotate-half), so the
        # table stays index-aligned and only carries rotate-half's sign
        sin64 = sin_b.T  # [64, S]
        sh = np.empty_like(sin64)
        sh[0:32] = -sin64[0:32]
        sh[32:64] = sin64[32:64]
        sinT = np.tile(sh, (2, 1)).astype(bf16)  # [128, S]
        mask_b = attention_mask[b, 0]  # [S(q), S(k)]
        if causal:
            em = np.empty((S, 1024), np.float16)
            for qc in range(QC):
                blk = mask_b[qc * 512:(qc + 1) * 512,
                             qc * 512:(qc + 1) * 512].T  # [k, q]
                e = np.exp(blk).astype(np.float16)
                em[qc * 512:(qc + 1) * 512, 0:512] = e
                em[qc * 512:(qc + 1) * 512, 512:1024] = e
        else:
            em = np.exp(mask_b.T).astype(np.float16)  # [k, q]
        for g in range(CPB):
            c0 = g * HPC * HD
            in_maps.append({
                "hsT": hsT_b, "cosk": cosT, "sink": sinT, "emask": em,
                "wq": np.ascontiguousarray(wq_f[:, c0:c0 + HPC * HD]).astype(bf16),
                "wk": np.ascontiguousarray(wk_f[:, c0:c0 + HPC * HD]).astype(bf16),
                "wv": np.ascontiguousarray(wv_f[:, c0:c0 + HPC * HD]).astype(bf16),
                "wo": wo_,
            })
    return in_maps


def run(inputs: dict, trace: bool = False):
    causal = _detect_causal(inputs["attention_mask"])
    nc = _get_program(causal)
    in_maps = make_in_maps(**inputs, causal=causal)
    res = run_bass_kernel_spmd(nc, in_maps, list(range(N_CORES)), trace=trace)
    out = np.empty((B, S, HID), np.float32)
    for c in range(N_CORES):
        b, r = c // CPB, c % CPB
        for qc in range(QC):
            q0 = qc * 512 + r * 128
            out[b, q0:q0 + 128, :] = res.results[c]["out"][qc * 128:(qc + 1) * 128]
    return out, res


def kernel(**inputs) -> np.ndarray:
    out, _ = run(inputs, trace=False)
    return out


# revision 37
# speedup vs baseline: 1.1035x; 1.0687x over previous
"""Multi-head attention (RoPE + causal-mask softmax) on 8 TRN2 NeuronCores.

Sharding: batch x head-group (2 batches x 4 groups of 4 heads). Each core
computes q/k/v projections for its 4 heads over the full sequence and
attention for all 2048 queries. Per 512-query chunk, an AllGather over the
4 cores of the batch exchanges normalized attention outputs; each core then
reads its rank's 128-query column window (runtime-offset DMA) and runs the
full 16-head o_proj locally, so no partial-sum reduction is needed.

Head-sharding keeps the program SPMD-uniform while letting the causal
structure skip score blocks above the block diagonal (every core sees the
same query/key trapezoid). kernel() inspects the mask at runtime: if it is
(effectively) causal it builds the trapezoid program, otherwise a full-mask
fallback program.
"""

from contextlib import ExitStack

import numpy as np

import concourse.bass as bass
import concourse.tile as tile
from concourse import bacc, mybir
from concourse.alu_op_type import AluOpType
from concourse.bass_utils import run_bass_kernel_spmd

AF = mybir.ActivationFunctionType
F32 = mybir.dt.float32
F16 = mybir.dt.float16
BF16 = mybir.dt.bfloat16

B, S, HID, NH, HD = 2, 2048, 1024, 16, 64
SCALE = 1.0 / np.sqrt(HD)
N_CORES = 8
HPC = 4            # heads per core
CPB = 4            # cores per batch
HC = HID // 128    # hidden chunks (8)
QC = S // 512      # query chunks of 512 (4)
KC = S // 128      # key chunks of 128 (16)
GROUPS = [[0, 1, 2, 3], [4, 5, 6, 7]]


def build_program(causal: bool):
    nc = bacc.Bacc("TRN2", target_bir_lowering=False, debug=False,
                   num_devices=N_CORES)

    hsT = nc.dram_tensor("hsT", [HID, S], BF16, kind="ExternalInput").ap()
    cosk = nc.dram_tensor("cosk", [128, S], BF16, kind="ExternalInput").ap()
    sink = nc.dram_tensor("sink", [128, S], BF16, kind="ExternalInput").ap()
    # causal: exp(mask) diag blocks, [keys 512 per qc stacked, q 512 x2 dup]
    # general: exp(mask) full, [keys S, q S]
    em_cols = 1024 if causal else S
    emask = nc.dram_tensor("emask", [S, em_cols], F16, kind="ExternalInput").ap()
    wq = nc.dram_tensor("wq", [HID, HPC * HD], BF16, kind="ExternalInput").ap()
    wk = nc.dram_tensor("wk", [HID, HPC * HD], BF16, kind="ExternalInput").ap()
    wv = nc.dram_tensor("wv", [HID, HPC * HD], BF16, kind="ExternalInput").ap()
    wo = nc.dram_tensor("wo", [HID, HID], F16, kind="ExternalInput").ap()
    out = nc.dram_tensor("out", [512, HID], F32, kind="ExternalOutput").ap()

    with tile.TileContext(nc) as tc, ExitStack() as top:
        res = top.enter_context(tc.tile_pool(name="res", bufs=1))
        dram = top.enter_context(tc.tile_pool(name="dram", bufs=1, space="DRAM"))

        # AllGather exchange buffers, one per query chunk: each core
        # contributes its normalized [2 pairs x 128, 512 q] block; after the
        # gather, rows [g*256 + p*128] hold peer g's pair-p heads and every
        # core reads its own 128-query column window (rank-dynamic offset).
        ag_in = [dram.tile([256, 512], F16, tag=f"ai{qc}", name=f"ai{qc}")
                 for qc in range(QC)]
        ag_out = [dram.tile([CPB * 256, 512], F16, tag=f"ao{qc}",
                            name=f"ao{qc}") for qc in range(QC)]
        agl_in = [dram.tile([128, 512], F16, tag=f"ali{p}", name=f"ali{p}")
                  for p in range(2)]
        agl_out = [dram.tile([CPB * 128, 512], F16, tag=f"alo{p}",
                             name=f"alo{p}") for p in range(2)]

        # ---- resident tiles, batched multi-dim DMA loads --------------------
        # hsT as one [128, HC*S] tile; per-seq-chunk loads spread across the
        # three DMA-capable queues so the K projection can start early.
        hsT_t = res.tile([128, HC * S], BF16, tag="hsT")
        hs3 = hsT_t[:].rearrange("p (c s) -> p c s", c=HC)
        wk_t = res.tile([128, HC * 256], BF16, tag="wk")
        nc.gpsimd.dma_start(wk_t[:].rearrange("p (c n) -> p c n", c=HC),
                            wk[:].rearrange("(c p) n -> p c n", c=HC))
        # first seq chunk split across queues so K proj starts early
        nc.sync.dma_start(
            hs3[:, 0:3, 0:512],
            hsT[0:384, 0:512].rearrange("(c p) q -> p c q", c=3))
        nc.scalar.dma_start(
            hs3[:, 3:6, 0:512],
            hsT[384:768, 0:512].rearrange("(c p) q -> p c q", c=3))
        nc.gpsimd.dma_start(
            hs3[:, 6:8, 0:512],
            hsT[768:1024, 0:512].rearrange("(c p) q -> p c q", c=2))
        for sc, eng in ((1, nc.sync), (2, nc.scalar), (3, nc.sync)):
            eng.dma_start(
                hs3[:, :, sc * 512:(sc + 1) * 512],
                hsT[:, sc * 512:(sc + 1) * 512].rearrange(
                    "(c p) q -> p c q", c=HC))

        def hsv(hc):
            return hsT_t[:, hc * S:(hc + 1) * S]

        wq_t = res.tile([128, HC * 256], BF16, tag="wq")
        nc.gpsimd.dma_start(wq_t[:].rearrange("p (c n) -> p c n", c=HC),
                            wq[:].rearrange("(c p) n -> p c n", c=HC))
        wv_t = res.tile([128, HC * 256], BF16, tag="wv")
        nc.gpsimd.dma_start(wv_t[:].rearrange("p (c n) -> p c n", c=HC),
                            wv[:].rearrange("(c p) n -> p c n", c=HC))
        cos_sb = res.tile([128, S], BF16, tag="cos")
        nc.scalar.dma_start(cos_sb[:], cosk[:])
        sin_sb = res.tile([128, S], BF16, tag="sin")
        nc.scalar.dma_start(sin_sb[:], sink[:])
        # K/Q pair-packed [head dims: pair head A 0:64, head B 64:128]
        kt = [res.tile([128, S], BF16, tag=f"kt{p}", name=f"kt{p}")
              for p in range(2)]
        qt = [res.tile([128, S], BF16, tag=f"qt{p}", name=f"qt{p}")
              for p in range(2)]
        # V augmented: per key-chunk, 4 heads x (64 cols + ones col)
        v_sb = [res.tile([128, HPC * 65], F16, tag=f"v{kc}", name=f"v{kc}")
                for kc in range(KC)]
        ones4 = res.tile([128, HPC], F16, tag="ones4")
        nc.gpsimd.memset(ones4[:], 1.0)
        # normalized attention output, pair-packed
        acc2 = [res.tile([128, S], F16, tag=f"acc2_{p}", name=f"acc2_{p}")
                for p in range(2)]

        # ---- K/Q projection + RoPE -----------------------------------------
        # Full-width rope per pair: project all 4 seq chunks into kraw, swap
        # the 32-row rotate-half blocks with 4 wide SBUF->SBUF DMAs, then 3
        # full-width vector ops. sin_sb carries rotate-half's sign.
        with tc.tile_pool(name="rope", bufs=2) as rope, \
             tc.tile_pool(name="psk", bufs=2, space="PSUM") as psk:
            for w_t, dst_l in ((wk_t, kt), (wq_t, qt)):
                for p in range(2):
                    with nc.allow_low_precision(reason="bf16 rope"):
                        kraw = rope.tile([128, S], BF16, tag="kraw")
                        for sc in range(4):
                            ps = psk.tile([128, 512], F32, tag="psk")
                            for hc in range(HC):
                                nc.tensor.matmul(
                                    ps[:],
                                    w_t[:, hc * 256 + p * 128:
                                        hc * 256 + (p + 1) * 128],
                                    hsv(hc)[:, sc * 512:(sc + 1) * 512],
                                    start=(hc == 0), stop=(hc == HC - 1))
                            nc.vector.tensor_copy(
                                kraw[:, sc * 512:(sc + 1) * 512], ps[:])
                        ksw = rope.tile([128, S], BF16, tag="ksw")
                        for hb in (0, 64):
                            nc.gpsimd.dma_start(ksw[hb:hb + 32, :],
                                                kraw[hb + 32:hb + 64, :])
                            nc.gpsimd.dma_start(ksw[hb + 32:hb + 64, :],
                                                kraw[hb:hb + 32, :])
                        t1 = rope.tile([128, S], BF16, tag="t1")
                        nc.vector.tensor_tensor(t1[:], kraw[:], cos_sb[:],
                                                AluOpType.mult)
                        t2 = rope.tile([128, S], BF16, tag="t2")
                        nc.vector.tensor_tensor(t2[:], ksw[:], sin_sb[:],
                                                AluOpType.mult)
                        nc.vector.tensor_tensor(dst_l[p][:], t1[:], t2[:],
                                                AluOpType.add)

        # ---- mid loads: wo + exp(mask), transfer during the rope window -----
        wo_t = res.tile([128, HC * HID], F16, tag="wo")
        nc.scalar.dma_start(wo_t[:].rearrange("p (c n) -> p c n", c=HC),
                            wo[:].rearrange("(c p) n -> p c n", c=HC))
        em_sb = []
        nkc_em = 4 if causal else KC
        em_w = 1024 if causal else 512
        for qc in range(QC):
            t = res.tile([128, nkc_em * em_w], F16, tag=f"em{qc}",
                         name=f"em{qc}")
            esrc = (emask[qc * 512:(qc + 1) * 512, :] if causal
                    else emask[:, qc * 512:(qc + 1) * 512])
            nc.scalar.dma_start(
                t[:].rearrange("p (c q) -> p c q", c=nkc_em),
                esrc.rearrange("(c p) q -> p c q", c=nkc_em))
            em_sb.append(t[:].rearrange("p (c q) -> p c q", c=nkc_em))

        # ---- V projection ---------------------------------------------------
        with tc.tile_pool(name="psv", bufs=2, space="PSUM") as psv:
            for kc in range(KC):
                ps = psv.tile([128, HPC * HD], F32, tag="psv")
                for hc in range(HC):
                    nc.tensor.matmul(
                        ps[:], hsv(hc)[:, kc * 128:(kc + 1) * 128],
                        wv_t[:, hc * 256:(hc + 1) * 256],
                        start=(hc == 0), stop=(hc == HC - 1))
                v3 = v_sb[kc][:].rearrange("p (h c) -> p h c", h=HPC)
                ps3 = ps[:].rearrange("p (h c) -> p h c", h=HPC)
                with nc.allow_low_precision(reason="fp16 v"):
                    nc.vector.tensor_copy(v3[:, :, 0:64], ps3[:])
                nc.gpsimd.tensor_copy(v3[:, :, 64], ones4[:])

        # ---- attention + exchange + o_proj, per query chunk -----------------
        with tc.tile_pool(name="expp", bufs=4) as expp, \
             tc.tile_pool(name="nrm", bufs=2) as nrm, \
             tc.tile_pool(name="gath", bufs=2) as gathp, \
             tc.tile_pool(name="outp", bufs=2) as outp, \
             tc.tile_pool(name="pss", bufs=2, space="PSUM") as pss, \
             tc.tile_pool(name="psa", bufs=1, space="PSUM") as psa, \
             tc.tile_pool(name="pso", bufs=1, space="PSUM") as pso:
            # our rank's query-column window within each gathered chunk
            col0 = (nc.sync.partition_id() % CPB) * 128
            cc_pending = []

            def flush_cc():
                while cc_pending:
                    q = cc_pending.pop(0)
                    nc.gpsimd.collective_compute(
                        "AllGather", AluOpType.bypass, replica_groups=GROUPS,
                        ins=[ag_in[q].opt()], outs=[ag_out[q].opt()])

            def oproj(qc):
                # logically delay past all attention so the scheduler cannot
                # hoist these ahead in the queues (they wait on a collective;
                # hoisting head-of-line-blocks the tensor queue behind it)
                ctx = tc.tile_wait_until(1.0 + 0.001 * qc)
                ctx.__enter__()
                gath = gathp.tile([128, 8 * 128], F16, tag="gath",
                                  name="gath")
                if qc < QC - 1:
                    # all 16 heads for our rank's 128 queries in one DMA
                    nc.sync.dma_start(
                        gath[:].rearrange("p (b c) -> p b c", b=8),
                        ag_out[qc][:].rearrange(
                            "(b p) q -> p b q",
                            b=8)[:, :, bass.ds(col0, 128)])
                else:
                    for p in range(2):
                        nc.sync.dma_start(
                            gath[:].rearrange("p (g t c) -> p t g c", g=CPB,
                                              t=2)[:, p, :, :],
                            agl_out[p][:].rearrange(
                                "(g x) q -> x g q",
                                g=CPB)[:, :, bass.ds(col0, 128)])
                t_out = outp.tile([128, 1024], F32, tag="tout", name="t_out")
                ps = pso.tile([128, 1024], F32, tag="pso", name="ps")
                # even head-blocks come from the pair-0 exchange, odd from
                # pair-1: accumulate evens first so only the last 4 matmuls
                # wait on the final collective
                hbs = [0, 2, 4, 6, 1, 3, 5, 7] if qc == QC - 1 else list(range(HC))
                for nn in range(2):
                    for i, hb in enumerate(hbs):
                        nc.tensor.matmul(
                            ps[:, nn * 512:(nn + 1) * 512],
                            gath[:, hb * 128:(hb + 1) * 128],
                            wo_t[:, hb * HID + nn * 512:
                                 hb * HID + (nn + 1) * 512],
                            start=(i == 0), stop=(i == HC - 1))
                    nc.vector.tensor_copy(
                        t_out[:, nn * 512:(nn + 1) * 512],
                        ps[:, nn * 512:(nn + 1) * 512])
                nc.sync.dma_start(out[qc * 128:(qc + 1) * 128, :], t_out[:])
                ctx.__exit__(None, None, None)

            for qc in range(QC):
                n_kc = 4 * (qc + 1) if causal else KC
                for p in range(2):
                    ps_a = [psa.tile([65, 512], F32, tag=f"psa{h}",
                                     name=f"psa{h}") for h in range(2)]
                    for kc in range(n_kc):
                        # on diagonal blocks only queries >= key block are
                        # live: restrict everything to q in [q0w, 512)
                        q0w = max(0, kc - 4 * qc) * 128 if causal else 0
                        pse = pss.tile([128, 1024], F32, tag="pse")
                        for half in range(2):
                            hb = half * 64
                            nc.tensor.matmul(
                                pse[:, half * 512 + q0w:(half + 1) * 512],
                                kt[p][hb:hb + 64, kc * 128:(kc + 1) * 128],
                                qt[p][hb:hb + 64,
                                      qc * 512 + q0w:(qc + 1) * 512],
                                start=True, stop=True)
                        tex = expp.tile([128, 1024], F16, tag="tex")
                        if q0w == 0:
                            nc.scalar.activation(tex[:], pse[:], AF.Exp)
                        else:
                            t3 = tex[:].rearrange("p (h q) -> p h q", h=2)
                            p3 = pse[:].rearrange("p (h q) -> p h q", h=2)
                            nc.scalar.activation(t3[:, :, q0w:],
                                                 p3[:, :, q0w:], AF.Exp)
                        if causal and kc >= 4 * qc:
                            tem = expp.tile([128, 1024], F16, tag="tem")
                            em2 = em_sb[qc][:, kc - 4 * qc, :]
                            for half in range(2):
                                nc.vector.tensor_tensor(
                                    tem[:, half * 512 + q0w:(half + 1) * 512],
                                    tex[:, half * 512 + q0w:(half + 1) * 512],
                                    em2[half * 512 + q0w:(half + 1) * 512]
                                    if False else
                                    em2[:, half * 512 + q0w:
                                        (half + 1) * 512],
                                    AluOpType.mult)
                        elif not causal:
                            tem = expp.tile([128, 1024], F16, tag="tem")
                            for half in range(2):
                                nc.vector.tensor_tensor(
                                    tem[:, half * 512:(half + 1) * 512],
                                    tex[:, half * 512:(half + 1) * 512],
                                    em_sb[qc][:, kc, :], AluOpType.mult)
                        else:
                            tem = tex
                        for half in range(2):
                            h = 2 * p + half
                            nc.tensor.matmul(
                                ps_a[half][:, q0w:512],
                                v_sb[kc][:, h * 65:h * 65 + 65],
                                tem[:, half * 512 + q0w:(half + 1) * 512],
                                start=(kc == 0), stop=(kc == n_kc - 1))
                    for half in range(2):
                        hb = half * 64
                        # den lives at PSUM partition 64; hop it to partition
                        # 0 (32-aligned cross-partition copy is legal), recip
                        # there, then broadcast (which always reads part. 0)
                        rec0 = nrm.tile([1, 512], F32, tag="rec0")
                        nc.vector.tensor_copy(rec0[:], ps_a[half][64:65, :])
                        rect = nrm.tile([1, 512], F32, tag="rect")
                        nc.vector.reciprocal_approx_fast(rect[:], rec0[:])
                        recb = nrm.tile([64, 512], F32, tag="recb")
                        nc.gpsimd.partition_broadcast(recb[:], rect[:])
                        with nc.allow_low_precision(reason="fp16 attn out"):
                            nc.vector.tensor_tensor(
                                acc2[p][hb:hb + 64, qc * 512:(qc + 1) * 512],
                                ps_a[half][0:64, :], recb[:], AluOpType.mult)
                    if qc == QC - 1:
                        nc.sync.dma_start(
                            agl_in[p][:], acc2[p][:, qc * 512:(qc + 1) * 512])
                        flush_cc()
                        nc.gpsimd.collective_compute(
                            "AllGather", AluOpType.bypass,
                            replica_groups=GROUPS,
                            ins=[agl_in[p].opt()], outs=[agl_out[p].opt()])
                # ship this chunk's normalized outputs: last chunk goes out
                # per pair (the pair-0 gather hides under pair-1 attention)
                if qc < QC - 1:
                    for p in range(2):
                        nc.sync.dma_start(
                            ag_in[qc][p * 128:(p + 1) * 128, :],
                            acc2[p][:, qc * 512:(qc + 1) * 512])
                    # defer the CC issue one chunk: the collective blocks
                    # the gpsimd queue while it runs, so issue it only after
                    # the NEXT chunk's broadcasts are already enqueued
                    flush_cc()
                    cc_pending.append(qc)
                # o_proj for the PREVIOUS chunk (its exchange overlapped this
                # chunk's attention) - keeps the tensor queue from stalling
                if qc > 1:
                    oproj(qc - 2)
            oproj(QC - 2)
            oproj(QC - 1)

    nc.compile()
    return nc


_NC_CACHE = {}


def _get_program(causal: bool = True):
    if causal not in _NC_CACHE:
        _NC_CACHE[causal] = build_program(causal)
    return _NC_CACHE[causal]


def _detect_causal(attention_mask):
    """True if everything at or above the 512-block diagonal's upper edge is
    masked off hard enough that exp(mask) == 0 for our purposes."""
    m = np.asarray(attention_mask)  # [B, 1, S(q), S(k)]
    for qc in range(QC):
        k0 = (qc + 1) * 512
        if k0 >= S:
            continue
        blk = m[:, 0, qc * 512:(qc + 1) * 512, k0:]
        if not np.all(blk < -30.0):
            return False
    return True


def make_in_maps(hidden_states, attention_mask, position_ids, cos, sin,
                 Wq, Wk, Wv, Wo, causal):
    import ml_dtypes
    bf16 = ml_dtypes.bfloat16
    hidden_states = np.asarray(hidden_states, np.float32)
    attention_mask = np.asarray(attention_mask, np.float32)
    position_ids = np.asarray(position_ids)
    cos = np.asarray(cos, np.float32)
    sin = np.asarray(sin, np.float32)
    wq_f = np.asarray(Wq, np.float32) * SCALE
    wk_f = np.asarray(Wk, np.float32)
    wv_f = np.asarray(Wv, np.float32)
    wo_ = np.ascontiguousarray(np.asarray(Wo, np.float32)).astype(np.float16)

    in_maps = []
    for b in range(B):
        hsT_b = np.ascontiguousarray(hidden_states[b].T).astype(bf16)
        cos_b = cos[position_ids[b]]  # [S, HD]
        sin_b = sin[position_ids[b]]
        cosT = np.tile(cos_b.T, (2, 1)).astype(bf16)  # [128, S]
        # signed sin: the device swaps k's 32-row halves (rotate-half), so the
        # table stays index-aligned and only carries rotate-half's sign
        sin64 = sin_b.T  # [64, S]
        sh = np.empty_like(sin64)
        sh[0:32] = -sin64[0:32]
        sh[32:64] = sin64[32:64]
        sinT = np.tile(sh, (2, 1)).astype(bf16)  # [128, S]
        mask_b = attention_mask[b, 0]  # [S(q), S(k)]
        if causal:
            em = np.empty((S, 1024), np.float16)
            for qc in range(QC):
                blk = mask_b[qc * 512:(qc + 1) * 512,
                             qc * 512:(qc + 1) * 512].T  # [k, q]
                e = np.exp(blk).astype(np.float16)
                em[qc * 512:(qc + 1) * 512, 0:512] = e
                em[qc * 512:(qc + 1) * 512, 512:1024] = e
        else:
            em = np.exp(mask_b.T).astype(np.float16)  # [k, q]
        for g in range(CPB):
            c0 = g * HPC * HD
            in_maps.append({
                "hsT": hsT_b, "cosk": cosT, "sink": sinT, "emask": em,
                "wq": np.ascontiguousarray(wq_f[:, c0:c0 + HPC * HD]).astype(bf16),
                "wk": np.ascontiguousarray(wk_f[:, c0:c0 + HPC * HD]).astype(bf16),
                "wv": np.ascontiguousarray(wv_f[:, c0:c0 + HPC * HD]).astype(bf16),
                "wo": wo_,
            })
    return in_maps


def run(inputs: dict, trace: bool = False):
    causal = _detect_causal(inputs["attention_mask"])
    nc = _get_program(causal)
    in_maps = make_in_maps(**inputs, causal=causal)
    res = run_bass_kernel_spmd(nc, in_maps, list(range(N_CORES)), trace=trace)
    out = np.empty((B, S, HID), np.float32)
    for c in range(N_CORES):
        b, r = c // CPB, c % CPB
        for qc in range(QC):
            q0 = qc * 512 + r * 128
            out[b, q0:q0 + 128, :] = res.results[c]["out"][qc * 128:(qc + 1) * 128]
    return out, res


def kernel(**inputs) -> np.ndarray:
    out, _ = run(inputs, trace=False)
    return out
